# revision 4
# baseline (speedup 1.0000x reference)
"""Trainium2 Bass kernel for nn_Attention_73581379715274.

GQA attention layer (B=1, S=2048, D=2048, H=32, KVH=8, HD=64) with RoPE,
causal mask, per-head FFN (Linear(64,64)+SiLU), and output projection.

Sharding (8 NeuronCores):
  - Tensor-parallel over heads: core c owns q-heads 4c..4c+3 and kv-head c
    (column-parallel wq/wk/wv).
  - wo is column-parallel: per-head FFN outputs (bf16 [256, 2048] per core,
    transposed layout) are AllGathered; each core then computes its own 256
    output columns. 8x less collective traffic than row-parallel all-reduce.

On-chip layout: feature dims live on partitions (transposed), so QK^T
produces scores^T directly, the softmax denominator comes free from a
ones-augmented V column in the PV matmul, and no probability transposes are
needed. x is transposed + cast to bf16 on the host (layout prep only).
"""
import sys

sys.path.insert(0, "/opt/trn_rl_repo")

import numpy as np
import ml_dtypes

import concourse.bass as bass
import concourse.tile as tile
import concourse.mybir as mybir
from concourse import bacc
from concourse.bass_utils import run_bass_kernel_spmd
from concourse.masks import make_identity

BF16 = ml_dtypes.bfloat16

N_CORES = 8
B, S, D = 1, 2048, 2048
H, KVH = 32, 8
HD = 64
HPC = H // N_CORES          # 4 q-heads per core
ECOLS = HPC * HD            # 256 output columns per core
S_CHUNK = 512
N_SCHUNK = S // S_CHUNK     # 4
KT = D // 128               # 16 k-tiles for the D contraction
ST = S // 128               # 16 sequence 128-tiles

_nc_cache = {}


def _pairswap_mask():
    m = []
    for i in range(0, 32, 2):
        m += [i + 1, i]
    return m


def build_nc(causal: bool, apply_mask_t: bool):
    f32, bf16 = mybir.dt.float32, mybir.dt.bfloat16
    nc = bacc.Bacc("TRN2", target_bir_lowering=False, debug=False,
                   num_devices=N_CORES)

    xT = nc.dram_tensor("xT", [D, S], bf16, kind="ExternalInput")
    # packed projection weights: [wq_c(256) | wk_c wk_c (128) | wv_c(64)]
    wp = nc.dram_tensor("wp", [D, 448], bf16, kind="ExternalInput")
    cos2 = nc.dram_tensor("cos2", [128, S], f32, kind="ExternalInput")
    sinsig = nc.dram_tensor("sinsig", [128, S], f32, kind="ExternalInput")
    fw_in = nc.dram_tensor("fw_in", [HD, HD], bf16, kind="ExternalInput")
    fb_in = nc.dram_tensor("fb_in", [HD, 1], f32, kind="ExternalInput")
    wo_c = nc.dram_tensor("wo_c", [D, ECOLS], bf16, kind="ExternalInput")
    use_maskt = apply_mask_t and not causal
    if use_maskt:
        maskT = nc.dram_tensor("maskT", [S, S], f32, kind="ExternalInput")
    out_c = nc.dram_tensor("out_c", [S, ECOLS], f32, kind="ExternalOutput")

    with tile.TileContext(nc) as tc:
        with (
            tc.tile_pool(name="persist", bufs=1) as persist,
            tc.tile_pool(name="dram", bufs=1, space="DRAM") as dram,
        ):
            # ---- persistent SBUF tensors ----
            qT = persist.tile([128, 2, S], bf16, name="qT")
            kkT = persist.tile([128, S], bf16, name="kkT")
            v_aug = persist.tile([128, ST, HD + 1], bf16, name="v_aug")
            fw_sb = persist.tile([HD, HD], bf16, name="fw_sb")
            fb_sb = persist.tile([HD, 1], f32, name="fb_sb")
            ones_col = persist.tile([1, HD], f32, name="ones_col")
            wo_sb = persist.tile([128, KT, ECOLS], bf16, name="wo_sb")
            ident = persist.tile([128, 128], f32, name="ident")
            make_identity(nc, ident[:])

            nc.gpsimd.dma_start(fw_sb[:], fw_in[:])
            nc.gpsimd.dma_start(fb_sb[:], fb_in[:])
            nc.gpsimd.dma_start(wo_sb[:], wo_c.rearrange("(kt p) e -> p kt e", p=128))
            nc.vector.memset(ones_col[:], 1.0)
            for t_idx in range(ST):
                nc.gpsimd.memset(v_aug[:, t_idx, HD:HD + 1], 1.0)

            ag_in = dram.tile([ECOLS, S], bf16, name="ag_in")
            ag_out = dram.tile([H * HD, S], bf16, addr_space="Shared",
                               name="ag_out")

            # ================= phase 1: projections + RoPE =================
            with (
                tc.tile_pool(name="xt", bufs=1) as xt_pool,
                tc.tile_pool(name="trig", bufs=1) as trig_pool,
                tc.tile_pool(name="wp_pool", bufs=1) as wp_pool,
                tc.tile_pool(name="pp_q", bufs=3, space="PSUM") as pp_q,
                tc.tile_pool(name="pp_v", bufs=2, space="PSUM") as pp_v,
                tc.tile_pool(name="vtr", bufs=2, space="PSUM") as vtr_ps,
                tc.tile_pool(name="rope_tmp", bufs=2) as rope_tmp,
                tc.tile_pool(name="vtmp", bufs=2) as vtmp_pool,
            ):
                x_sb = xt_pool.tile([128, KT, S], bf16, name="x_sb")
                nc.gpsimd.dma_start(x_sb[:], xT.rearrange("(kt p) s -> p kt s", p=128))
                wp_sb = wp_pool.tile([128, KT, 448], bf16, name="wp_sb")
                nc.gpsimd.dma_start(wp_sb[:], wp.rearrange("(kt p) j -> p kt j", p=128))
                cos_sb = trig_pool.tile([128, S], f32, name="cos_sb")
                sin_sb = trig_pool.tile([128, S], f32, name="sin_sb")
                nc.gpsimd.dma_start(cos_sb[:], cos2[:])
                nc.gpsimd.dma_start(sin_sb[:], sinsig[:])

                swap = _pairswap_mask()

                for ci in range(N_SCHUNK):
                    sl = bass.ts(ci, S_CHUNK)
                    # grouped projections: g=0,1 -> q head pairs, g=2 -> kk
                    for g in range(3):
                        ps = pp_q.tile([128, S_CHUNK], f32, name="projps",
                                       tag="projps")
                        for k in range(KT):
                            nc.tensor.matmul(
                                ps[:], wp_sb[:, k, bass.ts(g, 128)],
                                x_sb[:, k, sl],
                                start=(k == 0), stop=(k == KT - 1),
                            )
                        # RoPE: out = ps*cos2 + pairswap(ps)*sinsig
                        sw = rope_tmp.tile([128, S_CHUNK], f32, name="sw",
                                           tag="sw")
                        nc.vector.stream_shuffle(sw[:], ps[:], swap)
                        m1 = rope_tmp.tile([128, S_CHUNK], f32, name="m1",
                                           tag="m1")
                        nc.vector.tensor_mul(m1[:], ps[:], cos_sb[:, sl])
                        m2 = rope_tmp.tile([128, S_CHUNK], f32, name="m2",
                                           tag="m2")
                        nc.gpsimd.tensor_mul(m2[:], sw[:], sin_sb[:, sl])
                        if g < 2:
                            nc.vector.tensor_add(qT[:, g, sl], m1[:], m2[:])
                        else:
                            nc.vector.tensor_add(kkT[:, sl], m1[:], m2[:])

                    # V projection (64 cols) + PE transpose to [sk, d]
                    psv = pp_v.tile([64, S_CHUNK], f32, name="projpsv",
                                    tag="projpsv")
                    for k in range(KT):
                        nc.tensor.matmul(
                            psv[:], wp_sb[:, k, 384:448], x_sb[:, k, sl],
                            start=(k == 0), stop=(k == KT - 1),
                        )
                    vt = vtmp_pool.tile([64, S_CHUNK], f32, name="vt", tag="vt")
                    nc.scalar.copy(vt[:], psv[:])
                    for j in range(S_CHUNK // 128):
                        t_idx = ci * 4 + j
                        tp = vtr_ps.tile([128, 64], f32, name="vtp", tag="vtp")
                        nc.tensor.transpose(tp[:], vt[:, bass.ts(j, 128)],
                                            ident[0:HD, 0:HD])
                        nc.vector.tensor_copy(v_aug[:, t_idx, 0:HD], tp[:])

            # ================= phase 2: attention =================
            with (
                tc.tile_pool(name="qk_ps", bufs=3, space="PSUM") as qk_ps,
                tc.tile_pool(name="pv_ps", bufs=2, space="PSUM") as pv_ps,
                tc.tile_pool(name="recb_ps", bufs=1, space="PSUM") as recb_ps,
                tc.tile_pool(name="z_ps", bufs=2, space="PSUM") as z_ps,
                tc.tile_pool(name="exp_sb", bufs=6) as exp_sb,
                tc.tile_pool(name="attn_tmp", bufs=3) as attn_tmp,
                tc.tile_pool(name="mt_pool", bufs=4) as mt_pool,
            ):
                for hp in range(2):
                    for ci in range(N_SCHUNK):
                        sl = bass.ts(ci, S_CHUNK)
                        t_max = ci * 4 + 3 if causal else ST - 1
                        pv = [pv_ps.tile([HD + 1, S_CHUNK], f32,
                                         name=f"pv{half}", tag="pv")
                              for half in range(2)]
                        for t in range(t_max + 1):
                            kslice = bass.ts(t, 128)
                            if use_maskt:
                                mt = mt_pool.tile([128, S_CHUNK], f32,
                                                  name="mt", tag="mt")
                                nc.sync.dma_start(mt[:], maskT[kslice, sl])
                            exps = []
                            for half in range(2):
                                ps = qk_ps.tile([128, S_CHUNK], f32,
                                                name="qk", tag="qk")
                                nc.tensor.matmul(
                                    ps[:],
                                    kkT[bass.ds(64 * half, 64), kslice],
                                    qT[bass.ds(64 * half, 64), hp, sl],
                                    start=True, stop=True,
                                    tile_position=(64 * half, 0),
                                )
                                ex = exp_sb.tile([128, S_CHUNK], bf16,
                                                 name="ex", tag="exp")
                                if use_maskt:
                                    # ps = ps/8 + maskT, then exp
                                    nc.vector.scalar_tensor_tensor(
                                        ps[:], ps[:], 0.125, mt[:],
                                        op0=mybir.AluOpType.mult,
                                        op1=mybir.AluOpType.add)
                                    nc.scalar.activation(
                                        ex[:], ps[:],
                                        mybir.ActivationFunctionType.Exp,
                                        bias=0.0, scale=1.0)
                                else:
                                    nc.scalar.activation(
                                        ex[:], ps[:],
                                        mybir.ActivationFunctionType.Exp,
                                        bias=0.0, scale=0.125)
                                if causal:
                                    dcol = t * 128 - ci * S_CHUNK
                                    if dcol > 0:
                                        nc.gpsimd.memset(ex[:, 0:dcol], 0.0)
                                    if dcol >= 0:
                                        nc.gpsimd.affine_select(
                                            ex[:, bass.ds(dcol, 128)],
                                            ex[:, bass.ds(dcol, 128)],
                                            pattern=[[1, 128]],
                                            compare_op=mybir.AluOpType.is_ge,
                                            fill=0.0, base=0,
                                            channel_multiplier=-1)
                                exps.append(ex)
                            for half in range(2):
                                nc.tensor.matmul(
                                    pv[half][:], v_aug[:, t, :], exps[half][:],
                                    start=(t == 0), stop=(t == t_max),
                                )
                        # normalize + per-head FFN
                        for half in range(2):
                            head = hp * 2 + half
                            lrow = attn_tmp.tile([1, S_CHUNK], f32,
                                                 name="lrow", tag="lrow")
                            nc.vector.tensor_copy(lrow[:],
                                                  pv[half][HD:HD + 1, :])
                            rec = attn_tmp.tile([1, S_CHUNK], f32,
                                                name="rec", tag="rec")
                            scr = attn_tmp.tile([1, S_CHUNK], f32,
                                                name="scr", tag="scr")
                            nc.vector.reciprocal_approx_accurate(
                                rec[:], lrow[:], scr[:])
                            recb = recb_ps.tile([HD, S_CHUNK], f32,
                                                name="recb", tag="recb")
                            nc.tensor.matmul(recb[:], ones_col[:], rec[:],
                                             start=True, stop=True)
                            pvc = attn_tmp.tile([HD, S_CHUNK], f32,
                                                name="pvc", tag="pvc")
                            nc.scalar.copy(pvc[:], pv[half][0:HD, :])
                            od = attn_tmp.tile([HD, S_CHUNK], bf16,
                                               name="od", tag="od")
                            nc.vector.tensor_mul(od[:], pvc[:], recb[:])
                            z = z_ps.tile([HD, S_CHUNK], f32, name="z",
                                          tag="z")
                            nc.tensor.matmul(z[:], fw_sb[:], od[:],
                                             start=True, stop=True)
                            at = attn_tmp.tile([HD, S_CHUNK], bf16,
                                               name="at", tag="at")
                            nc.scalar.activation(
                                at[:], z[:], mybir.ActivationFunctionType.Silu,
                                bias=fb_sb[:], scale=1.0)
                            nc.sync.dma_start(
                                ag_in[bass.ts(head, HD), sl], at[:])

            # ================= phase 3: AllGather + wo =================
            nc.gpsimd.collective_compute(
                "AllGather", mybir.AluOpType.bypass,
                replica_groups=[list(range(N_CORES))],
                ins=[ag_in[:].opt()], outs=[ag_out[:].opt()],
            )

            with (
                tc.tile_pool(name="ag_sb", bufs=1) as ag_pool,
                tc.tile_pool(name="wo_ps", bufs=4, space="PSUM") as wo_ps,
                tc.tile_pool(name="out_pool", bufs=4) as out_pool,
            ):
                ag_sb = ag_pool.tile([128, KT, S], bf16, name="ag_sb")
                nc.gpsimd.dma_start(
                    ag_sb[:], ag_out[:].rearrange("(kt p) s -> p kt s", p=128))
                for st in range(ST):
                    ps = wo_ps.tile([128, ECOLS], f32, name="wops", tag="wops")
                    for k in range(KT):
                        nc.tensor.matmul(
                            ps[:], ag_sb[:, k, bass.ts(st, 128)], wo_sb[:, k, :],
                            start=(k == 0), stop=(k == KT - 1),
                        )
                    ob = out_pool.tile([128, ECOLS], f32, name="ob", tag="ob")
                    nc.scalar.copy(ob[:], ps[:])
                    nc.sync.dma_start(out_c[bass.ts(st, 128), :], ob[:])

    nc.finalize()
    return nc


def _host_prep(x, freqs_cos, freqs_sin, wq, wk, wv, wo, fw, fb):
    """Host-side layout prep (transposes, slicing, dtype casts only)."""
    x2 = np.asarray(x, dtype=np.float32).reshape(S, D)
    xT = np.ascontiguousarray(x2.T).astype(BF16)

    cosT = np.asarray(freqs_cos, np.float32).T          # [32, S]
    sinT = np.asarray(freqs_sin, np.float32).T
    cos64 = np.repeat(cosT, 2, axis=0)                  # [64, S]
    sin64 = np.repeat(sinT, 2, axis=0)
    sign = np.where((np.arange(HD) % 2) == 0, -1.0, 1.0).astype(np.float32)
    ss64 = sin64 * sign[:, None]
    cos2 = np.ascontiguousarray(np.tile(cos64, (2, 1)))     # [128, S]
    sinsig = np.ascontiguousarray(np.tile(ss64, (2, 1)))

    fwb = np.asarray(fw, np.float32).astype(BF16)           # [d, e] natural
    fbv = np.ascontiguousarray(np.asarray(fb, np.float32).reshape(HD, 1))

    wq_f = np.asarray(wq, np.float32)
    wk_f = np.asarray(wk, np.float32)
    wv_f = np.asarray(wv, np.float32)
    wo_f = np.asarray(wo, np.float32)

    in_maps = []
    for c in range(N_CORES):
        wq_c = wq_f[:, c * ECOLS:(c + 1) * ECOLS]
        wk_c = wk_f[:, c * HD:(c + 1) * HD]
        wv_c = wv_f[:, c * HD:(c + 1) * HD]
        wpk = np.concatenate([wq_c, wk_c, wk_c, wv_c], axis=1).astype(BF16)
        wo_cc = np.ascontiguousarray(
            wo_f[:, c * ECOLS:(c + 1) * ECOLS]).astype(BF16)
        in_maps.append({
            "xT": xT, "wp": np.ascontiguousarray(wpk), "cos2": cos2,
            "sinsig": sinsig, "fw_in": fwb, "fb_in": fbv, "wo_c": wo_cc,
        })
    return in_maps


def _classify_mask(mask):
    m = np.asarray(mask, np.float32)
    if not m.any():
        return "zeros"
    tril = np.tril(np.ones((S, S), dtype=bool))
    if np.all(m[tril] == 0.0) and np.all(m[~tril] <= -1e4):
        return "causal"
    return "generic"


def kernel(**inputs):
    x = inputs["x"]
    mask = inputs["mask"]
    kind = _classify_mask(mask)
    causal = kind == "causal"
    apply_mask_t = kind == "generic"

    key = (causal, apply_mask_t)
    if key not in _nc_cache:
        _nc_cache[key] = build_nc(causal, apply_mask_t)
    nc = _nc_cache[key]

    in_maps = _host_prep(
        x, inputs["freqs_cos"], inputs["freqs_sin"],
        inputs["wq"], inputs["wk"], inputs["wv"], inputs["wo"],
        inputs["fw"], inputs["fb"])
    if apply_mask_t:
        mT = np.ascontiguousarray(np.asarray(mask, np.float32).T)
        for m in in_maps:
            m["maskT"] = mT

    res = run_bass_kernel_spmd(nc, in_maps, core_ids=list(range(N_CORES)))
    out = np.concatenate([res.results[c]["out_c"] for c in range(N_CORES)],
                         axis=1)
    return out.reshape(B, S, D).astype(np.float32)
